# revision 31
# baseline (speedup 1.0000x reference)
"""Trainium2 Bass kernel for a 12-layer GRU LM (embed -> 12x GRU -> vocab decoder).

Lockstep layer-pipeline across cores, full batch per core, PE-dense waves.

Cores 0-5 each own TWO consecutive GRU layers (slots A/B); the full batch
(B=32) flows through a 6-stage pipeline in chunks of C=8 timesteps (22
lockstep waves). Cores 6-7 run the identical instruction stream (SPMD) on
zero weights; all 8 cores decode their 1/8 vocab shard of the final layer's
output (tensor-parallel decoder).

Schedule (evolved from a 5.06 ms baseline to ~3.9 ms):
  - Scan fully unrolled; one decoder PSUM-group (~2us of independent
    matmuls) after every scan step keeps the PE dense through the step's
    serial DVE/ACT chain (PE idle + HAM re-throttle to 1.2 GHz were the
    top costs in the baseline trace).
  - A/B scans staggered by 4 steps (giB computed in chunk halves) so two
    serial chains are in flight; the chain's inter-engine handoff latency,
    not the matmul stream (~3us/step), bounds a single chain.
  - The gi_rz addend is folded into PSUM by an identity matmul issued
    FIRST in each step's accumulation (a later start=True would clear
    has_written bank-wide and turn accumulates into overwrites); the
    sigmoid then reads PSUM directly.
  - Per-wave, per-half Shared DRAM AllGather outputs (single writer each
    -> fast HBM-HBM collective path; non-Shared outputs pay an ~18us local
    copy). The first-half AllGather launches mid-wave (after B3) and hides
    under the rest of the wave.
  - Core 0's pipeline input (the embedding chunk, gathered+transposed
    locally on every core) is blended into xin with a per-core 0/1 mask
    instead of bouncing through DRAM staging.
  - Indirect gathers land in contiguous SBUF temps (strided indirect-DMA
    destinations mis-lower -> NaN), then DVE-copy into xin halves.
  - 4 decoder groups are pinned after the second AllGather via a zero-mul
    data dependency on B7 so the Tile scheduler cannot hoist them earlier;
    they cover the collective's tail.

All GEMMs fp16 (fp32 PSUM); fp8 weights fail numerically for the recurrence
(rel err 0.09-0.5 vs the 2e-2 budget).
"""

import os
import sys

sys.path.insert(0, "/opt/trn_rl_repo")

import contextlib

import numpy as np

import concourse.bass as bass
import concourse.tile as tile
from concourse import bacc, mybir
from concourse.bass_utils import run_bass_kernel_spmd
from concourse.masks import make_identity

F32 = mybir.dt.float32
F16 = mybir.dt.float16
I32 = mybir.dt.int32

# Problem shapes (hardcoded per contract)
VOCAB, H, L, T, B = 30522, 768, 12, 128, 32
N_CORES = 8
JH = H // 128               # 6 feature chunks
G3 = 3 * H // 128           # 18 gate chunks
GRZ = 2 * H // 128          # 12 rz gate chunks
C = 8                       # timesteps per chunk
TOKC = C * B                # 256 tokens per chunk
NCHUNK = T // C             # 16 chunks
PIPE = 6                    # pipeline depth in cores (2 layers each)
WAVES = NCHUNK + PIPE       # 22 lockstep waves
VPAD = 30720                # vocab padded to 8 * 3840
VS = VPAD // N_CORES        # 3840 vocab shard per core
VC = 480                    # decoder psum chunk (8 per shard)
NTOK = T * B                # 4096 tokens
SROWS = N_CORES * 128 + 128  # staging rows: 8 AG slots + embed slot
SCOLS = JH * TOKC           # 1536

_CACHE = {}


def _build():
    nc = bacc.Bacc("TRN2", target_bir_lowering=False, debug=False,
                   num_devices=N_CORES)

    emb = nc.dram_tensor("emb", [VOCAB, H], F16, kind="ExternalInput").ap()
    idsq = nc.dram_tensor("idsq", [128, 2 * NCHUNK], I32, kind="ExternalInput").ap()
    wihT = nc.dram_tensor("wihT", [2, JH, 128, 3 * H], F16, kind="ExternalInput").ap()
    whhT = nc.dram_tensor("whhT", [2, JH, 128, 3 * H], F16, kind="ExternalInput").ap()
    wib = nc.dram_tensor("wib", [2, 1, 3 * H], F16, kind="ExternalInput").ap()
    bhhn = nc.dram_tensor("bhhn", [2, 128, JH], F16, kind="ExternalInput").ap()
    decT = nc.dram_tensor("decT", [JH, 128, VS], F16, kind="ExternalInput").ap()
    decb = nc.dram_tensor("decb", [1, VS], F16, kind="ExternalInput").ap()
    srcidx = nc.dram_tensor("srcidx", [128, 1], I32, kind="ExternalInput").ap()
    keep = nc.dram_tensor("keep", [WAVES, 128, JH * B], F16, kind="ExternalInput").ap()
    msel = nc.dram_tensor("msel", [128, 2, JH * C * B], F16, kind="ExternalInput").ap()
    out = nc.dram_tensor("out", [NTOK + TOKC, VS], F16, kind="ExternalOutput").ap()

    with tile.TileContext(nc) as tc, contextlib.ExitStack() as ctx:
        const = ctx.enter_context(tc.tile_pool(name="const", bufs=1))
        wpool = ctx.enter_context(tc.tile_pool(name="wpool", bufs=1))
        gpool = ctx.enter_context(tc.tile_pool(name="gpool", bufs=1))
        spool = ctx.enter_context(tc.tile_pool(name="spool", bufs=1))
        xpool = ctx.enter_context(tc.tile_pool(name="xpool", bufs=1))
        dpool = ctx.enter_context(tc.tile_pool(name="dpool", bufs=3))
        epool = ctx.enter_context(tc.tile_pool(name="epool", bufs=2))
        stpool = ctx.enter_context(tc.tile_pool(name="stpool", bufs=3))
        ps = ctx.enter_context(tc.tile_pool(name="ps", bufs=2, space="PSUM"))
        psg = ctx.enter_context(tc.tile_pool(name="psg", bufs=2, space="PSUM"))
        psd = ctx.enter_context(tc.tile_pool(name="psd", bufs=1, space="PSUM"))
        pse = ctx.enter_context(tc.tile_pool(name="pse", bufs=1, space="PSUM"))
        dram = ctx.enter_context(tc.tile_pool(name="dram", bufs=1, space="DRAM"))

        # per-wave AllGather output tensors, split in chunk halves so the
        # first collective launches mid-wave (after B3) and hides under the
        # rest of the wave. Shared (single writer each) = fast HBM-HBM path.
        SCOLS2 = SCOLS // 2
        Sag_a, Sag_b = [], []
        for w in range(WAVES):
            Swa = dram.tile([N_CORES * 128, SCOLS2], F16, addr_space="Shared",
                            name=f"Saga{w}")
            Swb = dram.tile([N_CORES * 128, SCOLS2], F16, addr_space="Shared",
                            name=f"Sagb{w}")
            Sag_a.append(Swa)
            Sag_b.append(Swb)
        SZ = dram.tile([N_CORES * 128, SCOLS2], F16)
        agin_a = [dram.tile([128, SCOLS2], F16, name=f"agina{i}")
                  for i in range(2)]
        agin_b = [dram.tile([128, SCOLS2], F16, name=f"aginb{i}")
                  for i in range(2)]

        # ---- constants / weights ----
        ident = const.tile([128, 128], F16)
        make_identity(nc, ident[:])
        ones = const.tile([1, TOKC], F16)
        nc.vector.memset(ones[:], 1.0)
        ids_sb = const.tile([128, 2 * NCHUNK], I32)
        nc.sync.dma_start(ids_sb[:], idsq[:])
        srcidx_sb = const.tile([128, 1], I32)
        nc.sync.dma_start(srcidx_sb[:], srcidx[:])

        wih_sb = const.tile([128, 2, JH, 3 * H], F16)
        whh_sb = const.tile([128, 2, JH, 3 * H], F16)
        wib_sb = const.tile([1, 2, 3 * H], F16)
        for s in range(2):
            for j in range(JH):
                nc.sync.dma_start(wih_sb[:, s, j, :], wihT[s, j])
                nc.sync.dma_start(whh_sb[:, s, j, :], whhT[s, j])
            nc.sync.dma_start(wib_sb[:, s, :], wib[s])
        bhhn_sb = wpool.tile([128, 2, JH, 1], F16)
        bhhn_bc = wpool.tile([128, 2, JH, B], F16)
        for s in range(2):
            nc.sync.dma_start(bhhn_sb[:, s, :, 0], bhhn[s])
            for b in range(B):
                nc.vector.tensor_copy(bhhn_bc[:, s, :, b:b + 1], bhhn_sb[:, s])
        decb_sb = const.tile([1, VS], F16)
        nc.sync.dma_start(decb_sb[:], decb[:])
        msel_sb = const.tile([128, 2, JH * C * B], F16)
        nc.sync.dma_start(msel_sb[:].rearrange("p a c -> p (a c)"),
                          msel[:].rearrange("p a c -> p (a c)"))

        # zero the wave-0 source so garbage waves stay finite
        zt = wpool.tile([128, SCOLS2], F16)
        nc.vector.memset(zt[:], 0.0)
        for r in range(N_CORES):
            nc.sync.dma_start(SZ[r * 128:(r + 1) * 128, :], zt[:])

        # ---- working tiles ----
        xin = xpool.tile([128, JH, C, B], F16)        # received chunk
        xtmp = xpool.tile([128, JH, C, B], F16)       # embed-blend temp
        xoutA = xpool.tile([128, JH, C, B], F16)
        xoutB = xpool.tile([128, JH, C, B], F16)
        xdec = xpool.tile([128, JH, TOKC], F16)
        xdec2 = xpool.tile([128, JH, TOKC], F16)      # post-AG dec copy (pin)
        zsmall = xpool.tile([128, JH, B], F16)
        nc.vector.memset(zsmall[:], 0.0)
        embfm = xpool.tile([128, JH, TOKC], F16)
        giA = gpool.tile([128, G3, C, B], F16)
        giB = gpool.tile([128, G3, C, B], F16)
        keep_sb = spool.tile([128, JH, B], F16)

        # per-slot chain tiles (slot on a free axis; A/B slices never conflict)
        rzs = spool.tile([128, 2, GRZ, B], F16)
        ghn = spool.tile([128, 2, JH, B], F16)
        npre = spool.tile([128, 2, JH, B], F16)
        nt = spool.tile([128, 2, JH, B], F16)
        dd = spool.tile([128, 2, JH, B], F16)
        zd = spool.tile([128, 2, JH, B], F16)
        nc.vector.memset(xoutA[:], 0.0)
        nc.vector.memset(xoutB[:], 0.0)

        xouts = [xoutA, xoutB]
        gis = [giA, giB]

        def gi_slot(s, gi, x, t0=0, t1=C):
            # gi[g, t, b] = sum_j wih[s]_j_g^T x_j + bias, tokens t0*B..t1*B
            ntok = (t1 - t0) * B
            xf = x[:, :, t0:t1, :].rearrange("p j t b -> p j (t b)")
            gf = gi[:, :, t0:t1, :].rearrange("p g t b -> p g (t b)")
            for g in range(G3):
                pg = psg.tile([128, TOKC], F32, tag="gips")
                for j in range(JH):
                    nc.tensor.matmul(pg[:, 0:ntok],
                                     wih_sb[:, s, j, g * 128:(g + 1) * 128],
                                     xf[:, j, :], start=(j == 0), stop=False)
                nc.tensor.matmul(pg[:, 0:ntok],
                                 wib_sb[0:1, s, g * 128:(g + 1) * 128],
                                 ones[0:1, 0:ntok], start=False, stop=True)
                nc.vector.tensor_copy(gf[:, g, :], pg[:, 0:ntok])

        def scan_step(s, gi, xout, t):
            # h_prev: previous timestep's output (t-1), or last step of the
            # previous wave's chunk (masked at wave start) for t == 0.
            # The gi_rz addend is folded into PSUM by one identity matmul so
            # the sigmoid reads PSUM directly (drops a DVE op + an engine
            # handoff from the serial chain).
            hp = xout[:, :, (t - 1) % C, :]
            pgh_rz = ps.tile([128, GRZ, B], F32, tag="ghps_rz")
            pgh_n = ps.tile([128, JH, B], F32, tag="ghps_n")
            # identity-MM FIRST: start=True writes gi_rz to the whole tile
            # (a later start would clear has_written bank-wide and turn the
            # accumulation into an overwrite)
            nc.tensor.matmul(pgh_rz[:], ident[:], gi[:, 0:GRZ, t, :],
                             start=True, stop=False)
            for g in range(GRZ):  # r,z gates accumulate onto gi_rz
                for j in range(JH):
                    nc.tensor.matmul(pgh_rz[:, g, :],
                                     whh_sb[:, s, j, g * 128:(g + 1) * 128],
                                     hp[:, j, :],
                                     start=False,
                                     stop=(g == GRZ - 1 and j == JH - 1))
            for g in range(GRZ, G3):  # n gates
                for j in range(JH):
                    nc.tensor.matmul(pgh_n[:, g - GRZ, :],
                                     whh_sb[:, s, j, g * 128:(g + 1) * 128],
                                     hp[:, j, :],
                                     start=(j == 0), stop=(j == JH - 1))
            # serial chain: sigmoid straight from PSUM
            nc.scalar.activation(rzs[:, s], pgh_rz[:],
                                 mybir.ActivationFunctionType.Sigmoid)
            nc.vector.tensor_add(ghn[:, s], pgh_n[:], bhhn_bc[:, s])
            nc.vector.tensor_mul(npre[:, s], rzs[:, s, 0:JH, :], ghn[:, s])
            nc.vector.tensor_add(npre[:, s], npre[:, s], gi[:, GRZ:G3, t, :])
            nc.scalar.activation(nt[:, s], npre[:, s],
                                 mybir.ActivationFunctionType.Tanh)
            nc.vector.tensor_sub(dd[:, s], hp, nt[:, s])
            nc.vector.tensor_mul(zd[:, s], rzs[:, s, JH:GRZ, :], dd[:, s])
            nc.vector.tensor_add(xout[:, :, t, :], zd[:, s], nt[:, s])

        def dec_group(vc, tg, orow, src=None):
            # one decoder psum group: ~1.8us of PE work, fills a chain gap
            src = xdec if src is None else src
            dwt = dec_w_tiles[vc]
            pd = psd.tile([128, VC], F32, tag="decps")
            for j in range(JH):
                nc.tensor.matmul(pd[:], src[:, j, tg * 128:(tg + 1) * 128],
                                 dwt[:, j, :], start=(j == 0), stop=False)
            nc.tensor.matmul(pd[:], ones[0:1, 0:128],
                             decb_sb[0:1, vc * VC:(vc + 1) * VC],
                             start=False, stop=True)
            stage = stpool.tile([128, VC], F16, tag="stage")
            nc.vector.tensor_copy(stage[:], pd[:])
            nc.sync.dma_start(
                out[orow + tg * 128:orow + (tg + 1) * 128,
                    vc * VC:(vc + 1) * VC], stage[:])

        def dec_load(vc):
            dwt = dpool.tile([128, JH, VC], F16, tag="decw")
            for j in range(JH):
                nc.sync.dma_start(dwt[:, j, :],
                                  decT[j, :, vc * VC:(vc + 1) * VC])
            return dwt

        def embed_gather(chunk, Sdst):
            # indirect-gather 2x128 token embeddings; transposes issued
            # separately (embed_transposes) so they can fill PE gaps
            tiles = []
            for grp in range(2):
                g = epool.tile([128, H], F16, tag="egather")
                col = chunk * 2 + grp
                nc.gpsimd.indirect_dma_start(
                    out=g[:], out_offset=None, in_=emb[:],
                    in_offset=bass.IndirectOffsetOnAxis(
                        ap=ids_sb[:, col:col + 1], axis=0),
                )
                tiles.append(g)
            return tiles

        def embed_transposes(tiles):
            # embfm stays SBUF-resident; core 0 blends it into xin next wave
            for grp in range(2):
                g = tiles[grp]
                for j in range(JH):
                    tp = pse.tile([128, 128], F16, tag="trps")
                    nc.tensor.transpose(out=tp[:], in_=g[:, j * 128:(j + 1) * 128],
                                        identity=ident[:])
                    nc.vector.tensor_copy(
                        embfm[:, j, grp * 128:(grp + 1) * 128], tp[:])

        # prologue: chunk 0 embedding (blended into xin by core 0 at wave 0)
        etiles = embed_gather(0, None)
        embed_transposes(etiles)

        for w in range(WAVES):
            Sca = Sag_a[w - 1] if w > 0 else SZ
            Scb = Sag_b[w - 1] if w > 0 else SZ

            # ---- embedding gather for chunk w+1 (DMA only; transposes later)
            etiles = embed_gather(min(w + 1, NCHUNK - 1), None)

            # ---- recv x chunk halves (per-core source row indices) ----
            # contiguous gather dests, then DVE copies into xin's halves
            rta = epool.tile([128, JH, 4 * B], F16, tag="rta")
            rtb = epool.tile([128, JH, 4 * B], F16, tag="rtb")
            nc.gpsimd.indirect_dma_start(
                out=rta[:].rearrange("p j tb -> p (j tb)"),
                out_offset=None, in_=Sca[:],
                in_offset=bass.IndirectOffsetOnAxis(ap=srcidx_sb[:, 0:1], axis=0),
            )
            nc.gpsimd.indirect_dma_start(
                out=rtb[:].rearrange("p j tb -> p (j tb)"),
                out_offset=None, in_=Scb[:],
                in_offset=bass.IndirectOffsetOnAxis(ap=srcidx_sb[:, 0:1], axis=0),
            )
            nc.vector.tensor_copy(
                xin[:, :, 0:4, :].rearrange("p j t b -> p j (t b)"), rta[:])
            nc.vector.tensor_copy(
                xin[:, :, 4:C, :].rearrange("p j t b -> p j (t b)"), rtb[:])
            # blend: core 0 takes the locally-computed embedding chunk instead
            # (msel col 0 is 1.0 on core 0, col 1 is the complement)
            xf = xin[:].rearrange("p j t b -> p (j t b)")
            ef = embfm[:].rearrange("p j t -> p (j t)")
            tf = xtmp[:].rearrange("p j t b -> p (j t b)")
            nc.vector.tensor_mul(tf, ef, msel_sb[:, 0, :])
            nc.vector.tensor_mul(xf, xf, msel_sb[:, 1, :])
            nc.vector.tensor_add(xf, xf, tf)

            # ---- dec input for this wave (layer-11 output, lagged PIPE waves)
            nc.sync.dma_start(xdec[:, :, 0:128], Sca[5 * 128:6 * 128, :])
            nc.sync.dma_start(xdec[:, :, 128:TOKC], Scb[5 * 128:6 * 128, :])
            rc = w - PIPE
            orow = rc * TOKC if 0 <= rc < NCHUNK else NTOK

            # ---- keep-mask: zero recurrent state outside the real window ----
            nc.sync.dma_start(keep_sb[:].rearrange("p j b -> p (j b)"), keep[w])
            for s in range(2):
                nc.vector.tensor_mul(xouts[s][:, :, C - 1, :],
                                     xouts[s][:, :, C - 1, :], keep_sb[:])

            # prefetch first dec weight chunks
            dec_w_tiles = {}
            dec_w_tiles[0] = dec_load(0)
            dec_w_tiles[1] = dec_load(1)

            dec_seq = [(vc, tg) for vc in range(VS // VC) for tg in range(2)]
            di = 0

            def dec_next(n, src=None):
                nonlocal di
                for _ in range(n):
                    vc, tg = dec_seq[di]
                    dec_group(vc, tg, orow, src=src)
                    di += 1
                    if tg == 1 and vc + 2 <= VS // VC - 1:
                        dec_w_tiles[vc + 2] = dec_load(vc + 2)

            # ---- staggered A/B scans: B lags A by 4 steps within the wave
            # so two serial chains are in flight; one dec group after every
            # step keeps the PE dense through the chain waits (HAM warm) ----
            gi_slot(0, giA, xin)
            for t in range(4):
                scan_step(0, giA, xoutA, t)
                dec_next(1)
            gi_slot(1, giB, xoutA, 0, 4)
            for t in range(4):
                scan_step(0, giA, xoutA, 4 + t)
                scan_step(1, giB, xoutB, t)
                dec_next(1)
            gi_slot(1, giB, xoutA, 4, C)

            # ---- first-half AllGather launches mid-wave (hidden) ----
            aga = agin_a[w % 2]
            nc.sync.dma_start(aga[:].rearrange("p (j tb) -> p j tb", j=JH),
                              xoutB[:, :, 0:4, :].rearrange("p j t b -> p j (t b)"))
            nc.gpsimd.collective_compute(
                "AllGather", mybir.AluOpType.bypass,
                replica_groups=[list(range(N_CORES))],
                ins=[aga.opt()], outs=[Sag_a[w][:]],
            )

            for t in range(4):
                scan_step(1, giB, xoutB, 4 + t)
                dec_next(1)

            # ---- second-half AllGather ----
            agb = agin_b[w % 2]
            nc.sync.dma_start(agb[:].rearrange("p (j tb) -> p j tb", j=JH),
                              xoutB[:, :, 4:C, :].rearrange("p j t b -> p j (t b)"))
            nc.gpsimd.collective_compute(
                "AllGather", mybir.AluOpType.bypass,
                replica_groups=[list(range(N_CORES))],
                ins=[agb.opt()], outs=[Sag_b[w][:]],
            )

            # ---- pinned post-AG cover: xdec2 gains a data-dependency on B7
            # via the zero-mul trick so the scheduler cannot hoist these dec
            # groups earlier; they execute during the AG_b flight ----
            nc.vector.tensor_copy(xdec2[:], xdec[:])
            nc.vector.tensor_mul(keep_sb[:], xouts[1][:, :, C - 1, :],
                                 zsmall[:])
            nc.vector.tensor_add(xdec2[:, :, 0:B], xdec2[:, :, 0:B],
                                 keep_sb[:])
            dec_next(len(dec_seq) - di, src=xdec2)
            embed_transposes(etiles)

    nc.compile()
    return nc


def _prep_inputs(input_ids, embedding, w_ih, w_hh, b_ih, b_hh, dec_w, dec_b):
    f16 = np.float16
    emb_np = np.ascontiguousarray(embedding.astype(f16))

    # ids: [T, B] -> [128, 2*NCHUNK]; col = chunk*2+grp, row p = token grp*128+p
    ids32 = np.asarray(input_ids).astype(np.int32).reshape(NCHUNK, C * B)
    idsq_np = np.ascontiguousarray(
        ids32.reshape(NCHUNK, 2, 128).transpose(2, 0, 1).reshape(128, 2 * NCHUNK))

    wihT_all = w_ih.transpose(0, 2, 1).reshape(L, JH, 128, 3 * H).astype(f16)
    whhT_all = w_hh.transpose(0, 2, 1).reshape(L, JH, 128, 3 * H).astype(f16)
    wib_all = b_ih.copy()
    wib_all[:, :2 * H] += b_hh[:, :2 * H]
    wib_all = wib_all.reshape(L, 1, 3 * H).astype(f16)
    bhhn_all = b_hh[:, 2 * H:].reshape(L, JH, 128).transpose(0, 2, 1).astype(f16)

    decT_full = np.zeros((JH, 128, VPAD), dtype=f16)
    decT_full[:, :, :VOCAB] = dec_w.T.reshape(JH, 128, VOCAB).astype(f16)
    decb_full = np.zeros((1, VPAD), dtype=f16)
    decb_full[0, :VOCAB] = dec_b.astype(f16)

    in_maps = []
    for c in range(N_CORES):
        if c < PIPE:
            l0, l1 = 2 * c, 2 * c + 1
            wih_np = np.ascontiguousarray(wihT_all[[l0, l1]])
            whh_np = np.ascontiguousarray(whhT_all[[l0, l1]])
            wib_np = np.ascontiguousarray(wib_all[[l0, l1]])
            bhhn_np = np.ascontiguousarray(bhhn_all[[l0, l1]])
        else:
            wih_np = np.zeros((2, JH, 128, 3 * H), dtype=f16)
            whh_np = np.zeros((2, JH, 128, 3 * H), dtype=f16)
            wib_np = np.zeros((2, 1, 3 * H), dtype=f16)
            bhhn_np = np.zeros((2, 128, JH), dtype=f16)

        base = 7 * 128 if c == 0 else (c - 1) * 128
        srcidx_np = (base + np.arange(128, dtype=np.int32)).reshape(128, 1)

        keep_np = np.zeros((WAVES, 128, JH * B), dtype=f16)
        if c < PIPE:
            keep_np[c + 1:c + NCHUNK] = 1.0

        msel_np = np.zeros((128, 2, JH * C * B), dtype=f16)
        msel_np[:, 0 if c == 0 else 1, :] = 1.0

        m = {
            "emb": emb_np, "idsq": idsq_np,
            "wihT": wih_np, "whhT": whh_np, "wib": wib_np, "bhhn": bhhn_np,
            "decT": np.ascontiguousarray(decT_full[:, :, c * VS:(c + 1) * VS]),
            "decb": np.ascontiguousarray(decb_full[:, c * VS:(c + 1) * VS]),
            "srcidx": srcidx_np, "keep": keep_np, "msel": msel_np,
        }
        in_maps.append(m)
    return in_maps


def kernel(input_ids, embedding, w_ih, w_hh, b_ih, b_hh, dec_w, dec_b):
    if "nc" not in _CACHE:
        _CACHE["nc"] = _build()
    nc = _CACHE["nc"]
    in_maps = _prep_inputs(input_ids, embedding, w_ih, w_hh, b_ih, b_hh,
                           dec_w, dec_b)
    res = run_bass_kernel_spmd(nc, in_maps, core_ids=list(range(N_CORES)))
    full = np.empty((T, B, VPAD), dtype=np.float32)
    for c in range(N_CORES):
        o = res.results[c]["out"][:NTOK].astype(np.float32)
        full[:, :, c * VS:(c + 1) * VS] = o.reshape(T, B, VS)
    return full[:, :, :VOCAB]


if __name__ == "__main__":
    _build()
    print("build OK")
